# revision 1
# baseline (speedup 1.0000x reference)
import time
from contextlib import ExitStack

import numpy as np

import concourse.bass as bass
import concourse.mybir as mybir
import concourse.tile as tile
from concourse import bacc
from concourse import bass_utils
from concourse._compat import axon_active
from concourse.bass import IndirectOffsetOnAxis

P = 128
F32 = mybir.dt.float32
BF16 = mybir.dt.bfloat16
I32 = mybir.dt.int32

N_NODES = 200000
F_IN = 165
HIDDEN = 384
F_OUT = 2
N_CORES = 8
TILES_PER_CORE = 196

LAST_RESULTS = None


def _ceil_div(a, b):
    return (a + b - 1) // b



def _preprocess(edge_index, n_nodes, n_cores, tiles_per_core):
    n_bins = n_cores * tiles_per_core
    n_pad = n_bins * P
    assert n_pad >= n_nodes

    row = np.asarray(edge_index[0], dtype=np.int64)
    col = np.asarray(edge_index[1], dtype=np.int64)
    loops = np.arange(n_nodes, dtype=np.int64)
    row_all = np.concatenate([row, loops])
    col_all = np.concatenate([col, loops])
    deg = np.bincount(col_all, minlength=n_nodes).astype(np.float64)
    dinv = 1.0 / np.sqrt(deg)
    nrm_all = (dinv[row_all] * dinv[col_all]).astype(np.float32)

    d = np.zeros(n_pad, np.int64)
    d[:n_nodes] = deg.astype(np.int64)
    order = np.argsort(-d, kind="stable")
    rows_idx = np.arange(n_pad) // n_bins
    pos = np.arange(n_pad) % n_bins
    bins_of_rank = np.where(rows_idx % 2 == 0, pos, n_bins - 1 - pos)
    bin_of_node = np.empty(n_pad, np.int64)
    slot_of_node = np.empty(n_pad, np.int64)
    bin_of_node[order] = bins_of_rank
    slot_of_node[order] = rows_idx
    perm = bin_of_node * P + slot_of_node

    load = np.bincount(bin_of_node[:n_nodes], weights=deg, minlength=n_bins)
    K = int(np.ceil(load.max() / P))

    e_src_dev = perm[row_all]
    e_dst_dev = perm[col_all]
    e_bin = e_dst_dev // P
    e_slot = e_dst_dev % P
    eo = np.argsort(e_bin, kind="stable")
    e_bin_s = e_bin[eo]
    cnt = np.bincount(e_bin_s, minlength=n_bins)
    starts = np.concatenate([[0], np.cumsum(cnt)[:-1]])
    within = np.arange(len(e_bin_s)) - starts[e_bin_s]
    lane = within % P
    chunk_global = e_bin_s * K + within // P

    n_ch = n_bins * K
    src_t = np.zeros((n_ch, P), np.int32)
    dst_t = np.zeros((n_ch, P), np.float32)
    nrm_t = np.zeros((n_ch, P), np.float32)
    src_t[chunk_global, lane] = e_src_dev[eo].astype(np.int32)
    dst_t[chunk_global, lane] = e_slot[eo].astype(np.float32)
    nrm_t[chunk_global, lane] = nrm_all[eo]

    TK = tiles_per_core * K
    src = src_t.reshape(n_cores, TK, P).transpose(0, 2, 1).copy()
    dstf = dst_t.reshape(n_cores, TK, P).transpose(0, 2, 1).copy()
    nrm = nrm_t.reshape(n_cores, TK, P).transpose(0, 2, 1).copy()
    return dict(perm=perm, K=K, src=src, dstf=dstf, nrm=nrm, n_pad=n_pad)


def _pack_w(W, f_out):
    f_in = W.shape[0]
    kc = _ceil_div(f_in, P)
    Wp = np.zeros((kc * P, f_out), np.float32)
    Wp[:f_in] = np.asarray(W, np.float32)
    return Wp.reshape(kc, P, f_out).transpose(1, 0, 2).reshape(P, kc * f_out).copy()


def _pack_b(b):
    f_out = b.shape[0]
    npj = _ceil_div(f_out, P)
    bp = np.zeros(npj * P, np.float32)
    bp[:f_out] = np.asarray(b, np.float32)
    return bp.reshape(npj, P).T.copy()



def _build_gcn(tc, ins, out_ap, cfg, sim_single_core=False):
    nc = tc.nc
    n_cores = cfg["n_cores"]
    T, K = cfg["T"], cfg["K"]
    F1, H, O = cfg["F1"], cfg["H"], cfg["O"]
    n_pad = n_cores * T * P
    kc1 = _ceil_div(F1, P)
    kc2 = _ceil_div(H, P)
    npj = _ceil_div(H, P)
    NCH = T * K
    rg = [list(range(n_cores))]
    BANK = 512

    ctx = ExitStack()
    with ctx:
        const = ctx.enter_context(tc.tile_pool(name="const", bufs=1))
        dram = ctx.enter_context(tc.tile_pool(name="dram", bufs=1, space="DRAM"))
        work = ctx.enter_context(tc.tile_pool(name="work", bufs=3))
        psum = ctx.enter_context(tc.tile_pool(name="psum", bufs=2, space="PSUM"))

        def load_const(name, shape, dtype=F32):
            t = const.tile(list(shape), dtype, name=name)
            nc.sync.dma_start(out=t[:], in_=ins[name][:])
            return t

        iota_sb = load_const("iota", [P, P])
        ident_sb = load_const("ident", [P, P])
        ident_bf = const.tile([P, P], BF16, name="ident_bf")
        nc.vector.tensor_copy(out=ident_bf[:], in_=ident_sb[:])
        w3_bf = const.tile([P, 3 * 2], BF16, name="w3_bf")
        w1_sb = load_const("w1", [P, kc1 * H])
        b1_sb = load_const("b1", [P, npj])
        w2_sb = load_const("w2", [P, kc2 * H])
        b2_sb = load_const("b2", [P, npj])
        w3_sb = load_const("w3", [P, kc2 * O])
        nc.vector.tensor_copy(out=w3_bf[:, :kc2 * O], in_=w3_sb[:])
        b3_sb = load_const("b3row", [P, O])
        src_sb = load_const("src", [P, NCH], I32)
        dstf_sb = load_const("dstf", [P, NCH])
        nrm_sb = load_const("nrm", [P, NCH])

        outbuf = const.tile([P, T * O], F32, name="outbuf")

        aspace = "Shared" if n_cores > 4 else "Local"
        x2s = dram.tile([T * P, H], BF16, name="x2s")
        x2f = dram.tile([n_pad, H], BF16, name="x2f", addr_space=aspace)
        h3s = dram.tile([T * P, O], BF16, name="h3s")
        h3f = dram.tile([n_pad, O], BF16, name="h3f", addr_space=aspace)

        Fmax = max(F1, H)

        xexp3 = ins["xexp"][:].rearrange("(c p) f -> p c f", p=P)

        def spmm_tile(t, x_src_ap, F, kc, dt, expanded=False):
            hgs, mhs = [], []
            hgall = None
            if expanded:
                hgall = work.tile([P, K * F], dt, name="hgall", tag="hgall",
                                  bufs=4)
                nc.sync.dma_start(
                    out=hgall[:].rearrange("p (c f) -> p c f", c=K),
                    in_=xexp3[:, t * K:(t + 1) * K, :])
            for c in range(K):
                ch = t * K + c
                if expanded:
                    hg = hgall[:, c * F:(c + 1) * F]
                    mh = work.tile([P, P], dt, name="mh", tag="mh", bufs=12)
                    nc.vector.tensor_scalar(
                        out=mh[:],
                        in0=iota_sb[:],
                        scalar1=dstf_sb[:, ch:ch + 1],
                        scalar2=nrm_sb[:, ch:ch + 1],
                        op0=mybir.AluOpType.is_equal,
                        op1=mybir.AluOpType.mult,
                    )
                    hgs.append(hg)
                    mhs.append(mh)
                    continue
                hg = work.tile([P, Fmax], dt, name="hg", tag="hg", bufs=12)
                nc.gpsimd.indirect_dma_start(
                    out=hg[:, :F],
                    out_offset=None,
                    in_=x_src_ap,
                    in_offset=IndirectOffsetOnAxis(ap=src_sb[:, ch:ch + 1], axis=0),
                )
                mh = work.tile([P, P], dt, name="mh", tag="mh", bufs=12)
                nc.vector.tensor_scalar(
                    out=mh[:],
                    in0=iota_sb[:],
                    scalar1=dstf_sb[:, ch:ch + 1],
                    scalar2=nrm_sb[:, ch:ch + 1],
                    op0=mybir.AluOpType.is_equal,
                    op1=mybir.AluOpType.mult,
                )
                hgs.append(hg)
                mhs.append(mh)
            aggT = work.tile([P, kc2 * P], F32, name="aggT", tag="aggT")
            for f in range(kc):
                fw = min(P, F - f * P)
                pa = psum.tile([P, P], F32, name="pa", tag="pa", bufs=2)
                for c in range(K):
                    nc.tensor.matmul(
                        out=pa[:fw, :],
                        lhsT=hgs[c][:, f * P:f * P + fw],
                        rhs=mhs[c][:],
                        start=(c == 0),
                        stop=(c == K - 1),
                    )
                nc.scalar.copy(out=aggT[:fw, f * P:f * P + P],
                               in_=pa[:fw, :])
            return aggT

        def transform_tile(t, aggT, F_in, kc_in, w_sb, b_sb, x_next_shard,
                           h3=False):
            pt = psum.tile([P, npj * P], F32, name="pt", tag="pt")
            for j in range(npj):
                jw = min(P, H - j * P)
                for k in range(kc_in):
                    kw = min(P, F_in - k * P)
                    nc.tensor.matmul(
                        out=pt[:jw, j * P:j * P + P],
                        lhsT=w_sb[:kw, k * H + j * P:k * H + j * P + jw],
                        rhs=aggT[:kw, k * P:k * P + P],
                        start=(k == 0),
                        stop=(k == kc_in - 1),
                    )
            xT = work.tile([P, npj * P], BF16, name="xT", tag="xT")
            for j in range(npj):
                jw = min(P, H - j * P)
                nc.scalar.activation(
                    out=xT[:jw, j * P:j * P + P],
                    in_=pt[:jw, j * P:j * P + P],
                    func=mybir.ActivationFunctionType.Relu,
                    bias=b_sb[:jw, j:j + 1],
                )
            if h3:
                h3p = psum.tile([P, P], F32, name="h3p", tag="ptp", bufs=3)
                for j in range(npj):
                    jw = min(P, H - j * P)
                    nc.tensor.matmul(
                        out=h3p[:, :O],
                        lhsT=xT[:jw, j * P:j * P + P],
                        rhs=w3_bf[:jw, j * O:(j + 1) * O],
                        start=(j == 0), stop=(j == npj - 1),
                    )
                h3row = work.tile([P, O], BF16, name="h3row", tag="h3row",
                                  bufs=3)
                nc.vector.tensor_copy(out=h3row[:], in_=h3p[:, :O])
                nc.sync.dma_start(out=x_next_shard[t * P:(t + 1) * P, :],
                                  in_=h3row[:])
                return
            xrow = work.tile([P, H], BF16, name="xrow", tag="xrow")
            for j in range(npj):
                jw = min(P, H - j * P)
                ptp = psum.tile([P, P], BF16, name="ptp", tag="ptp", bufs=3)
                nc.tensor.transpose(
                    out=ptp[:, :jw],
                    in_=xT[:jw, j * P:j * P + P],
                    identity=ident_bf[:jw, :jw],
                )
                nc.vector.tensor_copy(out=xrow[:, j * P:j * P + jw],
                                      in_=ptp[:, :jw])
            nc.sync.dma_start(out=x_next_shard[t * P:(t + 1) * P, :], in_=xrow[:])

        def allgather(src, dst):
            if sim_single_core:
                nc.sync.dma_start(out=dst[0:T * P, :], in_=src[:])
            else:
                nc.gpsimd.collective_compute(
                    "AllGather", mybir.AluOpType.bypass, replica_groups=rg,
                    ins=[src.opt()], outs=[dst.opt()],
                )

        for t in range(T):
            aggT = spmm_tile(t, ins["x"][:], F1, kc1, BF16, expanded=True)
            transform_tile(t, aggT, F1, kc1, w1_sb, b1_sb, x2s)
        allgather(x2s, x2f)

        for t in range(T):
            aggT = spmm_tile(t, x2f[:], H, kc2, BF16)
            transform_tile(t, aggT, H, kc2, w2_sb, b2_sb, h3s, h3=True)
        allgather(h3s, h3f)

        for t in range(T):
            po = psum.tile([P, P], F32, name="po", tag="ptp", bufs=3)
            for c in range(K):
                ch = t * K + c
                hg = work.tile([P, Fmax], BF16, name="hg", tag="hg", bufs=12)
                nc.gpsimd.indirect_dma_start(
                    out=hg[:, :O],
                    out_offset=None,
                    in_=h3f[:],
                    in_offset=IndirectOffsetOnAxis(ap=src_sb[:, ch:ch + 1],
                                                   axis=0),
                )
                mh = work.tile([P, P], BF16, name="mh", tag="mh", bufs=12)
                nc.vector.tensor_scalar(
                    out=mh[:],
                    in0=iota_sb[:],
                    scalar1=dstf_sb[:, ch:ch + 1],
                    scalar2=nrm_sb[:, ch:ch + 1],
                    op0=mybir.AluOpType.is_equal,
                    op1=mybir.AluOpType.mult,
                )
                nc.tensor.matmul(
                    out=po[:, :O],
                    lhsT=mh[:],
                    rhs=hg[:, :O],
                    start=(c == 0),
                    stop=(c == K - 1),
                )
            nc.vector.tensor_tensor(
                out=outbuf[:, t * O:(t + 1) * O],
                in0=po[:, :O],
                in1=b3_sb[:],
                op=mybir.AluOpType.add,
            )
        nc.sync.dma_start(out=out_ap, in_=outbuf[:])



EXEC_NS = None


def _run_pjrt_timed(nc, in_maps, n_cores, time_iters=0):
    global EXEC_NS
    import jax
    import jax.numpy as jnp
    from jax.experimental.shard_map import shard_map
    from jax.sharding import Mesh, NamedSharding, PartitionSpec

    from concourse import bass2jax as b2j

    b2j.install_neuronx_cc_hook()

    partition_name = (nc.partition_id_tensor.name
                      if nc.partition_id_tensor else None)
    in_names, out_names, out_avals, zero_outs = [], [], [], []
    for alloc in nc.m.functions[0].allocations:
        if not isinstance(alloc, mybir.MemoryLocationSet):
            continue
        name = alloc.memorylocations[0].name
        if alloc.kind == "ExternalInput":
            if name != partition_name:
                in_names.append(name)
        elif alloc.kind == "ExternalOutput":
            out_names.append(name)
            shape = tuple(alloc.tensor_shape)
            dtype = mybir.dt.np(alloc.dtype)
            out_avals.append(jax.core.ShapedArray(shape, dtype))
            zero_outs.append(np.zeros(shape, dtype))
    n_params = len(in_names)
    all_in_names = list(in_names) + list(out_names)
    if partition_name is not None:
        all_in_names.append(partition_name)
    all_in_names = tuple(all_in_names)

    def _body(*args):
        operands = list(args)
        if partition_name is not None:
            operands.append(b2j.partition_id_tensor())
        outs = b2j._bass_exec_p.bind(
            *operands,
            out_avals=tuple(out_avals),
            in_names=all_in_names,
            out_names=tuple(out_names),
            lowering_input_output_aliases=(),
            sim_require_finite=True,
            sim_require_nnan=True,
            nc=nc,
        )
        return tuple(outs)

    devices = jax.devices()[:n_cores]
    assert len(devices) == n_cores
    mesh = Mesh(np.asarray(devices), ("core",))
    spec = PartitionSpec("core")
    n_all = n_params + len(zero_outs)
    jitted = jax.jit(shard_map(
        _body, mesh=mesh, in_specs=(spec,) * n_all,
        out_specs=(spec,) * len(out_names), check_rep=False))

    sharding = NamedSharding(mesh, spec)
    g_in = [
        jax.device_put(
            np.concatenate([np.asarray(in_maps[c][nm]) for c in range(n_cores)],
                           axis=0), sharding)
        for nm in in_names
    ]
    g_zero = [
        jax.device_put(np.concatenate([z] * n_cores, axis=0), sharding)
        for z in zero_outs
    ]

    out_arrs = jitted(*g_in, *g_zero)
    jax.block_until_ready(out_arrs)
    results = [
        {nm: np.asarray(out_arrs[i]).reshape(n_cores, *out_avals[i].shape)[c]
         for i, nm in enumerate(out_names)}
        for c in range(n_cores)
    ]

    if time_iters > 0:
        triv = jax.jit(shard_map(
            lambda a: (a + 1.0,), mesh=mesh, in_specs=(spec,),
            out_specs=(spec,), check_rep=False))
        tiny = jax.device_put(np.zeros((n_cores * 8, 8), np.float32), sharding)
        jax.block_until_ready(triv(tiny))
        walls, base = [], []
        for _ in range(time_iters):
            t0 = time.perf_counter()
            o = jitted(*g_in, *g_zero)
            jax.block_until_ready(o)
            walls.append(time.perf_counter() - t0)
            t0 = time.perf_counter()
            o = triv(tiny)
            jax.block_until_ready(o)
            base.append(time.perf_counter() - t0)
        walls = np.array(walls)
        base = np.array(base)
        diffs = walls - base
        EXEC_NS = int((np.min(walls) - np.min(base)) * 1e9)
        print(f"[timing] kernel min {np.min(walls)*1e3:.3f} "
              f"med {np.median(walls)*1e3:.3f} ms | base min "
              f"{np.min(base)*1e3:.3f} med {np.median(base)*1e3:.3f} ms | "
              f"min-diff {EXEC_NS/1e3:.0f} us  med-diff "
              f"{np.median(diffs)*1e6:.0f} us")
    return results



def kernel(x, edge_index, W1, b1, W2, b2, W3, b3, _trace=False, _time_iters=0):
    global LAST_RESULTS
    x = np.asarray(x, np.float32)
    edge_index = np.asarray(edge_index)
    n_nodes = x.shape[0]
    assert n_nodes == N_NODES and x.shape[1] == F_IN

    pre = _preprocess(edge_index, n_nodes, N_CORES, TILES_PER_CORE)
    T, K = TILES_PER_CORE, pre["K"]
    n_pad = pre["n_pad"]
    cfg = dict(n_cores=N_CORES, T=T, K=K, F1=F_IN, H=HIDDEN, O=F_OUT)

    import ml_dtypes
    x_dev = np.zeros((n_pad, F_IN), ml_dtypes.bfloat16)
    x_dev[pre["perm"][:n_nodes]] = x
    TK = TILES_PER_CORE * pre["K"]

    common = dict(
        x=x_dev,
        iota=np.tile(np.arange(P, dtype=np.float32), (P, 1)).copy(),
        ident=np.eye(P, dtype=np.float32),
        w1=_pack_w(W1, HIDDEN), b1=_pack_b(b1),
        w2=_pack_w(W2, HIDDEN), b2=_pack_b(b2),
        w3=_pack_w(W3, F_OUT),
        b3row=np.tile(np.asarray(b3, np.float32), (P, 1)).copy(),
    )
    in_maps = []
    for c in range(N_CORES):
        m = dict(common)
        m["src"] = pre["src"][c]
        m["dstf"] = pre["dstf"][c]
        m["nrm"] = pre["nrm"][c]
        m["xexp"] = x_dev[pre["src"][c].T.reshape(-1)].copy()
        in_maps.append(m)

    nc = bacc.Bacc("TRN2", target_bir_lowering=False, debug=False,
                   enable_asserts=False, num_devices=N_CORES)
    ins_aps = {}
    for name, arr in in_maps[0].items():
        ins_aps[name] = nc.dram_tensor(
            name, list(arr.shape), mybir.dt.from_np(arr.dtype),
            kind="ExternalInput").ap()
    out_t = nc.dram_tensor("out", [P, T * F_OUT], F32, kind="ExternalOutput")

    with tile.TileContext(nc) as tc:
        _build_gcn(tc, ins_aps, out_t.ap(), cfg)
    nc.compile()

    if axon_active():
        results = _run_pjrt_timed(nc, in_maps, N_CORES, time_iters=_time_iters)
    else:
        res = bass_utils.run_bass_kernel_spmd(
            nc, in_maps, core_ids=list(range(N_CORES)), trace=_trace)
        LAST_RESULTS = res
        results = res.results

    out_dev = np.zeros((n_pad, F_OUT), np.float32)
    for c in range(N_CORES):
        o = results[c]["out"]
        rows = o.reshape(P, T, F_OUT).transpose(1, 0, 2).reshape(T * P, F_OUT)
        out_dev[c * T * P:(c + 1) * T * P] = rows
    return out_dev[pre["perm"][:n_nodes]].copy()



# revision 26
# speedup vs baseline: 1.2236x; 1.2236x over previous
import time
from contextlib import ExitStack

import numpy as np

import concourse.bass as bass
import concourse.mybir as mybir
import concourse.tile as tile
from concourse import bacc
from concourse import bass_utils
from concourse._compat import axon_active
from concourse.bass import IndirectOffsetOnAxis

P = 128
F32 = mybir.dt.float32
BF16 = mybir.dt.bfloat16
I32 = mybir.dt.int32

N_NODES = 200000
F_IN = 165
HIDDEN = 384
F_OUT = 2
N_CORES = 8
TILES_PER_CORE = 196

G1 = 4
G2 = 4
G3 = 14

LAST_RESULTS = None


def _ceil_div(a, b):
    return (a + b - 1) // b



def _preprocess(edge_index, n_nodes, n_cores, tiles_per_core):
    n_bins = n_cores * tiles_per_core
    n_pad = n_bins * P
    assert n_pad >= n_nodes

    row = np.asarray(edge_index[0], dtype=np.int64)
    col = np.asarray(edge_index[1], dtype=np.int64)
    loops = np.arange(n_nodes, dtype=np.int64)
    row_all = np.concatenate([row, loops])
    col_all = np.concatenate([col, loops])
    deg = np.bincount(col_all, minlength=n_nodes).astype(np.float64)
    dinv = 1.0 / np.sqrt(deg)
    nrm_all = (dinv[row_all] * dinv[col_all]).astype(np.float32)

    d = np.zeros(n_pad, np.int64)
    d[:n_nodes] = deg.astype(np.int64)
    order = np.argsort(-d, kind="stable")
    rows_idx = np.arange(n_pad) // n_bins
    pos = np.arange(n_pad) % n_bins
    bins_of_rank = np.where(rows_idx % 2 == 0, pos, n_bins - 1 - pos)
    bin_of_node = np.empty(n_pad, np.int64)
    slot_of_node = np.empty(n_pad, np.int64)
    bin_of_node[order] = bins_of_rank
    slot_of_node[order] = rows_idx
    perm = bin_of_node * P + slot_of_node

    load = np.bincount(bin_of_node[:n_nodes], weights=deg, minlength=n_bins)
    K = int(np.ceil(load.max() / P))

    e_src_dev = perm[row_all]
    e_dst_dev = perm[col_all]
    e_bin = e_dst_dev // P
    e_slot = e_dst_dev % P
    eo = np.argsort(e_bin, kind="stable")
    e_bin_s = e_bin[eo]
    cnt = np.bincount(e_bin_s, minlength=n_bins)
    starts = np.concatenate([[0], np.cumsum(cnt)[:-1]])
    within = np.arange(len(e_bin_s)) - starts[e_bin_s]
    lane = within % P
    chunk_global = e_bin_s * K + within // P

    n_ch = n_bins * K
    src_t = np.zeros((n_ch, P), np.int64)
    dst_t = np.zeros((n_ch, P), np.float32)
    nrm_t = np.zeros((n_ch, P), np.float32)
    src_t[chunk_global, lane] = e_src_dev[eo]
    dst_t[chunk_global, lane] = e_slot[eo].astype(np.float32)
    nrm_t[chunk_global, lane] = nrm_all[eo]

    T = tiles_per_core
    c_of = src_t // (T * P)
    r_of = src_t % (T * P)
    src3_t = (c_of * P + (r_of % P)) * T + (r_of // P)

    TK = tiles_per_core * K
    src = src_t.reshape(n_cores, TK, P).transpose(0, 2, 1).astype(np.int32).copy()
    src3 = src3_t.reshape(n_cores, TK, P).transpose(0, 2, 1).astype(np.int32).copy()
    dstf = dst_t.reshape(n_cores, TK, P).transpose(0, 2, 1).copy()
    nrm = nrm_t.reshape(n_cores, TK, P).transpose(0, 2, 1).copy()
    return dict(perm=perm, K=K, src=src, src3=src3, dstf=dstf, nrm=nrm,
                n_pad=n_pad)


def _pack_w(W, f_out, bias=None):
    import ml_dtypes
    f_in = W.shape[0]
    kc = _ceil_div(f_in, P)
    Wp = np.zeros((kc * P, f_out), np.float32)
    Wp[:f_in] = np.asarray(W, np.float32)
    if bias is not None:
        assert f_in % P != 0
        Wp[f_in] = np.asarray(bias, np.float32)
    return (Wp.reshape(kc, P, f_out).transpose(1, 0, 2)
            .reshape(P, kc * f_out).astype(ml_dtypes.bfloat16).copy())


def _pack_b(b):
    f_out = b.shape[0]
    npj = _ceil_div(f_out, P)
    bp = np.zeros(npj * P, np.float32)
    bp[:f_out] = np.asarray(b, np.float32)
    return bp.reshape(npj, P).T.copy()



def _build_gcn(tc, ins, out_ap, cfg, sim_single_core=False):
    nc = tc.nc
    n_cores = cfg["n_cores"]
    T, K = cfg["T"], cfg["K"]
    F1, H, O = cfg["F1"], cfg["H"], cfg["O"]
    n_pad = n_cores * T * P
    kc1 = _ceil_div(F1, P)
    kc2 = _ceil_div(H, P)
    npj = _ceil_div(H, P)
    NCH = T * K
    fw1 = F1 % P
    rg = [list(range(n_cores))]

    ctx = ExitStack()
    with ctx:
        const = ctx.enter_context(tc.tile_pool(name="const", bufs=1))
        dram = ctx.enter_context(tc.tile_pool(name="dram", bufs=1, space="DRAM"))
        work = ctx.enter_context(tc.tile_pool(name="work", bufs=3))
        psum = ctx.enter_context(tc.tile_pool(name="psum", bufs=2, space="PSUM"))

        def load_const(name, shape, dtype=F32):
            t = const.tile(list(shape), dtype, name=name)
            nc.sync.dma_start(out=t[:], in_=ins[name][:])
            return t

        iota_sb = load_const("iota", [P, P])
        w1_sb = load_const("w1", [P, kc1 * H], BF16)
        b1row_sb = load_const("b1row", [1, H], BF16)
        w2_sb = load_const("w2", [P, kc2 * H], BF16)
        b2_sb = load_const("b2", [P, npj])
        w3_sb = load_const("w3", [P, kc2 * O], BF16)
        ones_sb = const.tile([1, P], BF16, name="ones")
        nc.vector.memset(ones_sb[:], 1.0)
        b3_sb = load_const("b3row", [P, O])
        src_sb = load_const("src", [P, NCH], I32)
        src3_sb = load_const("src3", [P, NCH], I32)
        dstf_sb = load_const("dstf", [P, NCH])
        nrm_sb = load_const("nrm", [P, NCH])

        h3all = const.tile([P, T * O], BF16, name="h3all")
        outbuf = const.tile([P, T * O], F32, name="outbuf")

        aspace = "Local" if sim_single_core else "Shared"
        x2s = dram.tile([T * P, H], BF16, name="x2s")
        x2f = dram.tile([n_pad, H], BF16, name="x2f", addr_space=aspace)
        h3s = dram.tile([P, T * O], BF16, name="h3s")
        h3f = dram.tile([n_cores * P, T * O], BF16, name="h3f",
                        addr_space=aspace)

        xexp3 = ins["xexp"][:]

        def build_mh(ch, engine=None):
            eng = engine or nc.vector
            mh = work.tile([P, P], BF16, name="mh", tag="mh", bufs=12)
            eng.tensor_scalar(
                out=mh[:],
                in0=iota_sb[:],
                scalar1=dstf_sb[:, ch:ch + 1],
                scalar2=nrm_sb[:, ch:ch + 1],
                op0=mybir.AluOpType.is_equal,
                op1=mybir.AluOpType.mult,
            )
            return mh

        def allgather(src, dst):
            if sim_single_core:
                n = src.shape[0]
                for c in range(n_cores):
                    nc.sync.dma_start(out=dst[c * n:(c + 1) * n, :], in_=src[:])
            else:
                nc.gpsimd.collective_compute(
                    "AllGather", mybir.AluOpType.bypass, replica_groups=rg,
                    ins=[src.opt()], outs=[dst.opt()],
                )

        x2s3 = x2s[:].rearrange("(t p) f -> p t f", p=P)
        for tg in range(T // G1):
            t0 = tg * G1
            hgall = work.tile([P, G1 * K * F1], BF16, name="hg1", tag="hg1",
                              bufs=2)
            nc.sync.dma_start(
                out=hgall[:],
                in_=xexp3[:, t0 * K * F1:(t0 + G1) * K * F1])
            xgrp = work.tile([P, G1 * H], BF16, name="xgrp", tag="xgrp",
                             bufs=2)
            for ti in range(G1):
                t = t0 + ti
                pa = psum.tile([P, 4 * P], F32, name="pa", tag="pa", bufs=2)
                for c in range(K):
                    m = ti * K + c
                    mh = build_mh(t * K + c)
                    fs = range(kc1) if c < K - 1 else reversed(range(kc1))
                    for f in fs:
                        fw = min(P, F1 - f * P)
                        nc.tensor.matmul(
                            out=pa[:fw, f * P:(f + 1) * P],
                            lhsT=hgall[:, m * F1 + f * P:m * F1 + f * P + fw],
                            rhs=mh[:],
                            start=(c == 0 and f == 0),
                            stop=(c == K - 1 and f == 0),
                        )
                aggT = work.tile([P, kc1 * P], BF16, name="aggT1", tag="aggT1",
                                 bufs=2)
                nc.vector.tensor_copy(out=aggT[:, :P], in_=pa[:, :P])
                nc.vector.tensor_copy(out=aggT[:fw1, P:2 * P],
                                      in_=pa[:fw1, P:2 * P])
                prm = psum.tile([P, 4 * P], F32, name="prm", tag="pt", bufs=2)
                for k in range(kc1):
                    kw = min(P, F1 - k * P)
                    nc.tensor.matmul(
                        out=prm[:, :H],
                        lhsT=aggT[:kw, k * P:(k + 1) * P],
                        rhs=w1_sb[:kw, k * H:(k + 1) * H],
                        start=(k == 0),
                        stop=False,
                    )
                nc.tensor.matmul(
                    out=prm[:, :H],
                    lhsT=ones_sb[:, :],
                    rhs=b1row_sb[:, :],
                    start=False,
                    stop=True,
                )
                nc.scalar.activation(
                    out=xgrp[:, ti * H:(ti + 1) * H],
                    in_=prm[:, :H],
                    func=mybir.ActivationFunctionType.Relu,
                )
            nc.sync.dma_start(out=x2s3[:, t0:t0 + G1, :],
                              in_=xgrp[:].rearrange("p (t f) -> p t f", t=G1))
        allgather(x2s, x2f)
        dbg = cfg.get("dbg_aps") or {}
        if "dx2s" in dbg:
            nc.sync.dma_start(out=dbg["dx2s"], in_=x2s[:])
        if cfg.get("stages", 3) == 1:
            nc.sync.dma_start(out=out_ap, in_=outbuf[:])
            return

        for t in range(T):
            hgs = []
            for c in range(K):
                hg = work.tile([P, H], BF16, name="hg2", tag="hg2", bufs=8)
                nc.gpsimd.indirect_dma_start(
                    out=hg[:],
                    out_offset=None,
                    in_=x2f[:],
                    in_offset=IndirectOffsetOnAxis(
                        ap=src_sb[:, t * K + c:t * K + c + 1], axis=0),
                )
                hgs.append(hg)
            if True:
                pa = psum.tile([P, 4 * P], F32, name="pa2", tag="pa", bufs=2)
                for c in range(K):
                    mh = build_mh(t * K + c)
                    for f in range(kc2):
                        nc.tensor.matmul(
                            out=pa[:, f * P:(f + 1) * P],
                            lhsT=hgs[c][:, f * P:(f + 1) * P],
                            rhs=mh[:],
                            start=(c == 0 and f == 0),
                            stop=(c == K - 1 and f == kc2 - 1),
                        )
                aggT = work.tile([P, kc2 * P], BF16, name="aggT2", tag="aggT2",
                                 bufs=2)
                nc.scalar.copy(out=aggT[:], in_=pa[:, :kc2 * P])
                pt = psum.tile([P, 4 * P], F32, name="pt", tag="pt", bufs=2)
                for j in range(npj):
                    for k in range(kc2):
                        nc.tensor.matmul(
                            out=pt[:, j * P:(j + 1) * P],
                            lhsT=w2_sb[:, k * H + j * P:k * H + (j + 1) * P],
                            rhs=aggT[:, k * P:(k + 1) * P],
                            start=(j == 0 and k == 0),
                            stop=(j == npj - 1 and k == kc2 - 1),
                        )
                xT = work.tile([P, npj * P], BF16, name="xT", tag="xT", bufs=2)
                for j in range(npj - 1):
                    nc.scalar.activation(
                        out=xT[:, j * P:(j + 1) * P],
                        in_=pt[:, j * P:(j + 1) * P],
                        func=mybir.ActivationFunctionType.Relu,
                        bias=b2_sb[:, j:j + 1],
                    )
                j = npj - 1
                nc.vector.tensor_scalar(
                    out=xT[:, j * P:(j + 1) * P],
                    in0=pt[:, j * P:(j + 1) * P],
                    scalar1=b2_sb[:, j:j + 1],
                    scalar2=0.0,
                    op0=mybir.AluOpType.add,
                    op1=mybir.AluOpType.max,
                )
                h3p = psum.tile([P, 4 * P], F32, name="h3p", tag="h3p", bufs=2)
                for j in range(npj):
                    nc.tensor.matmul(
                        out=h3p[:, :O],
                        lhsT=xT[:, j * P:(j + 1) * P],
                        rhs=w3_sb[:, j * O:(j + 1) * O],
                        start=(j == 0), stop=(j == npj - 1),
                    )
                nc.scalar.copy(out=h3all[:, t * O:(t + 1) * O],
                               in_=h3p[:, :O])
        nc.sync.dma_start(out=h3s[:], in_=h3all[:])
        allgather(h3s, h3f)
        if "dh3s" in dbg:
            nc.sync.dma_start(out=dbg["dh3s"], in_=h3all[:])
        if cfg.get("stages", 3) == 2:
            nc.sync.dma_start(out=out_ap, in_=outbuf[:])
            return

        h3rows = h3f[:].rearrange("p (t o) -> (p t) o", o=O)
        for t in range(T):
            hg3s = []
            for c in range(K):
                hg3 = work.tile([P, O], BF16, name="hg3", tag="hg3", bufs=8)
                nc.gpsimd.indirect_dma_start(
                    out=hg3[:],
                    out_offset=None,
                    in_=h3rows,
                    in_offset=IndirectOffsetOnAxis(
                        ap=src3_sb[:, t * K + c:t * K + c + 1], axis=0),
                )
                hg3s.append(hg3)
            if True:
                po = psum.tile([P, 4 * P], F32, name="po", tag="h3p", bufs=2)
                for c in range(K):
                    mh = build_mh(t * K + c)
                    nc.tensor.matmul(
                        out=po[:, :O],
                        lhsT=mh[:],
                        rhs=hg3s[c][:],
                        start=(c == 0),
                        stop=(c == K - 1),
                    )
                nc.vector.tensor_tensor(
                    out=outbuf[:, t * O:(t + 1) * O],
                    in0=po[:, :O],
                    in1=b3_sb[:],
                    op=mybir.AluOpType.add,
                )
        nc.sync.dma_start(out=out_ap, in_=outbuf[:])



EXEC_NS = None


def _run_pjrt_timed(nc, in_maps, n_cores, time_iters=0):
    global EXEC_NS
    import jax
    import jax.numpy as jnp
    from jax.experimental.shard_map import shard_map
    from jax.sharding import Mesh, NamedSharding, PartitionSpec

    from concourse import bass2jax as b2j

    b2j.install_neuronx_cc_hook()

    partition_name = (nc.partition_id_tensor.name
                      if nc.partition_id_tensor else None)
    in_names, out_names, out_avals, zero_outs = [], [], [], []
    for alloc in nc.m.functions[0].allocations:
        if not isinstance(alloc, mybir.MemoryLocationSet):
            continue
        name = alloc.memorylocations[0].name
        if alloc.kind == "ExternalInput":
            if name != partition_name:
                in_names.append(name)
        elif alloc.kind == "ExternalOutput":
            out_names.append(name)
            shape = tuple(alloc.tensor_shape)
            dtype = mybir.dt.np(alloc.dtype)
            out_avals.append(jax.core.ShapedArray(shape, dtype))
            zero_outs.append(np.zeros(shape, dtype))
    n_params = len(in_names)
    all_in_names = list(in_names) + list(out_names)
    if partition_name is not None:
        all_in_names.append(partition_name)
    all_in_names = tuple(all_in_names)

    def _body(*args):
        operands = list(args)
        if partition_name is not None:
            operands.append(b2j.partition_id_tensor())
        outs = b2j._bass_exec_p.bind(
            *operands,
            out_avals=tuple(out_avals),
            in_names=all_in_names,
            out_names=tuple(out_names),
            lowering_input_output_aliases=(),
            sim_require_finite=True,
            sim_require_nnan=True,
            nc=nc,
        )
        return tuple(outs)

    devices = jax.devices()[:n_cores]
    assert len(devices) == n_cores
    mesh = Mesh(np.asarray(devices), ("core",))
    spec = PartitionSpec("core")
    n_all = n_params + len(zero_outs)
    jitted = jax.jit(shard_map(
        _body, mesh=mesh, in_specs=(spec,) * n_all,
        out_specs=(spec,) * len(out_names), check_rep=False))

    sharding = NamedSharding(mesh, spec)
    g_in = [
        jax.device_put(
            np.concatenate([np.asarray(in_maps[c][nm]) for c in range(n_cores)],
                           axis=0), sharding)
        for nm in in_names
    ]
    g_zero = [
        jax.device_put(np.concatenate([z] * n_cores, axis=0), sharding)
        for z in zero_outs
    ]

    out_arrs = jitted(*g_in, *g_zero)
    jax.block_until_ready(out_arrs)
    results = [
        {nm: np.asarray(out_arrs[i]).reshape(n_cores, *out_avals[i].shape)[c]
         for i, nm in enumerate(out_names)}
        for c in range(n_cores)
    ]

    if time_iters > 0:
        triv = jax.jit(shard_map(
            lambda a: (a + 1.0,), mesh=mesh, in_specs=(spec,),
            out_specs=(spec,), check_rep=False))
        tiny = jax.device_put(np.zeros((n_cores * 8, 8), np.float32), sharding)
        jax.block_until_ready(triv(tiny))
        walls, base = [], []
        for _ in range(time_iters):
            t0 = time.perf_counter()
            o = jitted(*g_in, *g_zero)
            jax.block_until_ready(o)
            walls.append(time.perf_counter() - t0)
            t0 = time.perf_counter()
            o = triv(tiny)
            jax.block_until_ready(o)
            base.append(time.perf_counter() - t0)
        walls = np.array(walls)
        base = np.array(base)
        diffs = walls - base
        EXEC_NS = int((np.min(walls) - np.min(base)) * 1e9)
        print(f"[timing] kernel min {np.min(walls)*1e3:.3f} "
              f"med {np.median(walls)*1e3:.3f} ms | base min "
              f"{np.min(base)*1e3:.3f} med {np.median(base)*1e3:.3f} ms | "
              f"min-diff {EXEC_NS/1e3:.0f} us  med-diff "
              f"{np.median(diffs)*1e6:.0f} us")
    return results



def kernel(x, edge_index, W1, b1, W2, b2, W3, b3, _trace=False, _time_iters=0):
    global LAST_RESULTS
    x = np.asarray(x, np.float32)
    edge_index = np.asarray(edge_index)
    n_nodes = x.shape[0]
    assert n_nodes == N_NODES and x.shape[1] == F_IN

    pre = _preprocess(edge_index, n_nodes, N_CORES, TILES_PER_CORE)
    T, K = TILES_PER_CORE, pre["K"]
    n_pad = pre["n_pad"]
    cfg = dict(n_cores=N_CORES, T=T, K=K, F1=F_IN, H=HIDDEN, O=F_OUT)

    import ml_dtypes
    x_dev = np.zeros((n_pad, F_IN), ml_dtypes.bfloat16)
    x_dev[pre["perm"][:n_nodes]] = x

    common = dict(
        iota=np.tile(np.arange(P, dtype=np.float32), (P, 1)).copy(),
        w1=_pack_w(W1, HIDDEN),
        b1row=np.asarray(b1, np.float32).reshape(1, HIDDEN)
              .astype(ml_dtypes.bfloat16).copy(),
        b2=_pack_b(b2),
        w2=_pack_w(W2, HIDDEN),
        w3=_pack_w(W3, F_OUT),
        b3row=np.tile(np.asarray(b3, np.float32), (P, 1)).copy(),
    )
    in_maps = []
    for c in range(N_CORES):
        m = dict(common)
        m["src"] = pre["src"][c]
        m["src3"] = pre["src3"][c]
        m["dstf"] = pre["dstf"][c]
        m["nrm"] = pre["nrm"][c]
        m["xexp"] = x_dev[pre["src"][c]].reshape(P, -1).copy()
        in_maps.append(m)

    nc = bacc.Bacc("TRN2", target_bir_lowering=False, debug=False,
                   enable_asserts=False, num_devices=N_CORES)
    ins_aps = {}
    for name, arr in in_maps[0].items():
        ins_aps[name] = nc.dram_tensor(
            name, list(arr.shape), mybir.dt.from_np(arr.dtype),
            kind="ExternalInput").ap()
    out_t = nc.dram_tensor("out", [P, T * F_OUT], F32, kind="ExternalOutput")

    with tile.TileContext(nc) as tc:
        _build_gcn(tc, ins_aps, out_t.ap(), cfg)
    nc.compile()

    if axon_active():
        results = _run_pjrt_timed(nc, in_maps, N_CORES, time_iters=_time_iters)
    else:
        res = bass_utils.run_bass_kernel_spmd(
            nc, in_maps, core_ids=list(range(N_CORES)), trace=_trace)
        LAST_RESULTS = res
        results = res.results

    out_dev = np.zeros((n_pad, F_OUT), np.float32)
    for c in range(N_CORES):
        o = results[c]["out"]
        rows = o.reshape(P, T, F_OUT).transpose(1, 0, 2).reshape(T * P, F_OUT)
        out_dev[c * T * P:(c + 1) * T * P] = rows
    return out_dev[pre["perm"][:n_nodes]].copy()


# revision 34
# speedup vs baseline: 1.2867x; 1.0516x over previous
import time
from contextlib import ExitStack

import numpy as np

import concourse.bass as bass
import concourse.mybir as mybir
import concourse.tile as tile
from concourse import bacc
from concourse import bass_utils
from concourse._compat import axon_active
from concourse.bass import IndirectOffsetOnAxis

P = 128
F32 = mybir.dt.float32
BF16 = mybir.dt.bfloat16
I32 = mybir.dt.int32
I16 = mybir.dt.int16

N_NODES = 200000
F_IN = 165
HIDDEN = 384
F_OUT = 2
N_CORES = 8
TILES_PER_CORE = 196

G1 = 4
W_ROWS = 32768

LAST_RESULTS = None


def _ceil_div(a, b):
    return (a + b - 1) // b



def _preprocess(edge_index, n_nodes, n_cores, tiles_per_core):
    n_bins = n_cores * tiles_per_core
    n_pad = n_bins * P
    assert n_pad >= n_nodes

    row = np.asarray(edge_index[0], dtype=np.int64)
    col = np.asarray(edge_index[1], dtype=np.int64)
    loops = np.arange(n_nodes, dtype=np.int64)
    row_all = np.concatenate([row, loops])
    col_all = np.concatenate([col, loops])
    deg = np.bincount(col_all, minlength=n_nodes).astype(np.float64)
    dinv = 1.0 / np.sqrt(deg)
    nrm_all = (dinv[row_all] * dinv[col_all]).astype(np.float32)

    d = np.zeros(n_pad, np.int64)
    d[:n_nodes] = deg.astype(np.int64)
    order = np.argsort(-d, kind="stable")
    rows_idx = np.arange(n_pad) // n_bins
    pos = np.arange(n_pad) % n_bins
    bins_of_rank = np.where(rows_idx % 2 == 0, pos, n_bins - 1 - pos)
    bin_of_node = np.empty(n_pad, np.int64)
    slot_of_node = np.empty(n_pad, np.int64)
    bin_of_node[order] = bins_of_rank
    slot_of_node[order] = rows_idx
    perm = bin_of_node * P + slot_of_node

    load = np.bincount(bin_of_node[:n_nodes], weights=deg, minlength=n_bins)
    K = int(np.ceil(load.max() / P))

    e_src_dev = perm[row_all]
    e_dst_dev = perm[col_all]
    e_bin = e_dst_dev // P
    e_slot = e_dst_dev % P
    eo = np.argsort(e_bin, kind="stable")
    e_bin_s = e_bin[eo]
    cnt = np.bincount(e_bin_s, minlength=n_bins)
    starts = np.concatenate([[0], np.cumsum(cnt)[:-1]])
    within = np.arange(len(e_bin_s)) - starts[e_bin_s]
    lane = within % P
    chunk_global = e_bin_s * K + within // P

    n_ch = n_bins * K
    src_t = np.zeros((n_ch, P), np.int64)
    dst_t = np.zeros((n_ch, P), np.float32)
    nrm_t = np.zeros((n_ch, P), np.float32)
    src_t[chunk_global, lane] = e_src_dev[eo]
    dst_t[chunk_global, lane] = e_slot[eo].astype(np.float32)
    nrm_t[chunk_global, lane] = nrm_all[eo]

    T = tiles_per_core
    c_of = src_t // (T * P)
    r_of = src_t % (T * P)
    src3_t = (c_of * P + (r_of % P)) * T + (r_of // P)

    TK = tiles_per_core * K
    src = src_t.reshape(n_cores, TK, P).transpose(0, 2, 1).astype(np.int32).copy()
    src3 = src3_t.reshape(n_cores, TK, P).transpose(0, 2, 1).astype(np.int32).copy()
    dstf = dst_t.reshape(n_cores, TK, P).transpose(0, 2, 1).copy()
    nrm = nrm_t.reshape(n_cores, TK, P).transpose(0, 2, 1).copy()
    return dict(perm=perm, K=K, src=src, src3=src3, dstf=dstf, nrm=nrm,
                n_pad=n_pad)


def _preprocess_l3(edge_index, pre, n_nodes, n_cores, T):
    row = np.asarray(edge_index[0], dtype=np.int64)
    col = np.asarray(edge_index[1], dtype=np.int64)
    loops = np.arange(n_nodes, dtype=np.int64)
    row_all = np.concatenate([row, loops])
    col_all = np.concatenate([col, loops])
    deg = np.bincount(col_all, minlength=n_nodes).astype(np.float64)
    dinv = 1.0 / np.sqrt(deg)
    nrm_all = (dinv[row_all] * dinv[col_all]).astype(np.float32)
    perm = pre["perm"]
    s_dev = perm[row_all]
    d_dev = perm[col_all]

    c_src = s_dev // (T * P)
    r_src = s_dev % (T * P)
    src3 = (c_src * P + (r_src % P)) * T + (r_src // P)
    w_e = src3 // W_ROWS
    NROWS3 = n_cores * P * T
    NWIN = _ceil_div(NROWS3, W_ROWS)

    per_core = []
    cnt = np.zeros((n_cores, NWIN), np.int64)
    for c in range(n_cores):
        m = (d_dev // (T * P)) == c
        dl = d_dev[m] - c * T * P
        t_e = dl // P
        slot_e = dl % P
        s3 = src3[m]
        w = w_e[m]
        nr = nrm_all[m]
        order = np.lexsort((slot_e, t_e, w))
        per_core.append((s3[order], w[order], t_e[order], slot_e[order],
                         nr[order]))
        cnt[c] = np.bincount(w[order], minlength=NWIN)

    NW = [int(_ceil_div(cnt[:, w].max(), P) * P) for w in range(NWIN)]
    L = sum(NW)
    C = L // P

    tile_l = np.full((n_cores, C, P), -1, np.int64)
    slot_l = np.full((n_cores, C, P), -1, np.int64)
    nrm_l = np.zeros((n_cores, C, P), np.float32)
    idx_l = np.zeros((n_cores, L), np.int64)
    for c in range(n_cores):
        s3, w, t_e, slot_e, nr = per_core[c]
        off = 0
        posn = 0
        for wi in range(NWIN):
            n = int(cnt[c, wi])
            lanes = off + np.arange(n)
            idx_l[c, lanes] = s3[posn:posn + n] - wi * W_ROWS
            fl = lanes // P
            ll = lanes % P
            tile_l[c, fl, ll] = t_e[posn:posn + n]
            slot_l[c, fl, ll] = slot_e[posn:posn + n]
            nrm_l[c, fl, ll] = nr[posn:posn + n]
            posn += n
            off += NW[wi]

    spans = []
    for k in range(C):
        ts = np.unique(tile_l[:, k, :])
        spans.append([int(t) for t in ts if t >= 0])
    SP = sum(len(s) for s in spans)

    dstf3 = np.full((n_cores, P, SP), -1.0, np.float32)
    nrm3 = np.zeros((n_cores, P, SP), np.float32)
    j = 0
    for k in range(C):
        for ti in spans[k]:
            for c in range(n_cores):
                sel = tile_l[c, k] == ti
                dstf3[c, sel, j] = slot_l[c, k, sel].astype(np.float32)
                nrm3[c, sel, j] = nrm_l[c, k, sel]
            j += 1

    X = [nw // 16 for nw in NW]
    idx16 = np.zeros((n_cores, P, sum(X)), np.int16)
    for c in range(n_cores):
        off = 0
        xoff = 0
        for wi in range(NWIN):
            nw = NW[wi]
            if nw == 0:
                continue
            vals = idx_l[c, off:off + nw]
            idx16[c, :16, xoff:xoff + X[wi]] = (
                vals.reshape(X[wi], 16).T.astype(np.int16))
            off += nw
            xoff += X[wi]

    return dict(NW=NW, spans=spans, dstf3=dstf3, nrm3=nrm3, idx16=idx16,
                NROWS3=NROWS3)


def _pack_w(W, f_out):
    import ml_dtypes
    f_in = W.shape[0]
    kc = _ceil_div(f_in, P)
    Wp = np.zeros((kc * P, f_out), np.float32)
    Wp[:f_in] = np.asarray(W, np.float32)
    return (Wp.reshape(kc, P, f_out).transpose(1, 0, 2)
            .reshape(P, kc * f_out).astype(ml_dtypes.bfloat16).copy())


def _pack_b(b):
    f_out = b.shape[0]
    npj = _ceil_div(f_out, P)
    bp = np.zeros(npj * P, np.float32)
    bp[:f_out] = np.asarray(b, np.float32)
    return bp.reshape(npj, P).T.copy()



def _build_gcn(tc, ins, out_ap, cfg, sim_single_core=False):
    nc = tc.nc
    n_cores = cfg["n_cores"]
    T, K = cfg["T"], cfg["K"]
    F1, H, O = cfg["F1"], cfg["H"], cfg["O"]
    n_pad = n_cores * T * P
    kc1 = _ceil_div(F1, P)
    kc2 = _ceil_div(H, P)
    npj = _ceil_div(H, P)
    NCH = T * K
    rg = [list(range(n_cores))]

    ctx = ExitStack()
    with ctx:
        const = ctx.enter_context(tc.tile_pool(name="const", bufs=1))
        dram = ctx.enter_context(tc.tile_pool(name="dram", bufs=1, space="DRAM"))
        work = ctx.enter_context(tc.tile_pool(name="work", bufs=3))
        psum = ctx.enter_context(tc.tile_pool(name="psum", bufs=2, space="PSUM"))

        def load_const(name, shape, dtype=F32):
            t = const.tile(list(shape), dtype, name=name)
            nc.sync.dma_start(out=t[:], in_=ins[name][:])
            return t

        iota_sb = load_const("iota", [P, P])
        w1_sb = load_const("w1", [P, kc1 * H], BF16)
        b1row_sb = load_const("b1row", [1, H], BF16)
        w2_sb = load_const("w2", [P, kc2 * H], BF16)
        b2_sb = load_const("b2", [P, npj])
        w3_sb = load_const("w3", [P, kc2 * O], BF16)
        ones_sb = const.tile([1, P], BF16, name="ones")
        nc.vector.memset(ones_sb[:], 1.0)
        b3_sb = load_const("b3full", [P, T * O])
        src_sb = load_const("src", [P, NCH], I32)
        dstf_sb = load_const("dstf", [P, NCH])
        nrm_sb = load_const("nrm", [P, NCH])
        src3_sb = load_const("src3", [P, NCH], I32)

        h3all = const.tile([P, T * O], BF16, name="h3all")
        outbuf = const.tile([P, T * O], F32, name="outbuf")

        aspace = "Local" if sim_single_core else "Shared"
        x2s = dram.tile([T * P, H], BF16, name="x2s")
        x2f = dram.tile([n_pad, H], BF16, name="x2f", addr_space=aspace)
        h3s = dram.tile([P, T * O], BF16, name="h3s")
        h3f = dram.tile([n_cores * P, T * O], BF16, name="h3f",
                        addr_space=aspace)

        xexp3 = ins["xexp"][:]

        if cfg.get("stages", 3) == 0:
            nc.vector.memset(outbuf[:], 0.0)
            nc.sync.dma_start(out=out_ap, in_=outbuf[:])
            return

        def build_mh(ch, engine=None):
            eng = engine or nc.vector
            mh = work.tile([P, P], BF16, name="mh", tag="mh", bufs=12)
            eng.tensor_scalar(
                out=mh[:],
                in0=iota_sb[:],
                scalar1=dstf_sb[:, ch:ch + 1],
                scalar2=nrm_sb[:, ch:ch + 1],
                op0=mybir.AluOpType.is_equal,
                op1=mybir.AluOpType.mult,
            )
            return mh

        def allgather(src, dst):
            if sim_single_core:
                n = src.shape[0]
                for c in range(n_cores):
                    nc.sync.dma_start(out=dst[c * n:(c + 1) * n, :], in_=src[:])
            else:
                nc.gpsimd.collective_compute(
                    "AllGather", mybir.AluOpType.bypass, replica_groups=rg,
                    ins=[src.opt()], outs=[dst.opt()],
                )

        fw1 = F1 % P
        x2s3 = x2s[:].rearrange("(t p) f -> p t f", p=P)
        for tg in range(T // G1):
            t0 = tg * G1
            hgall = work.tile([P, G1 * K * F1], BF16, name="hg1", tag="hg1",
                              bufs=2)
            nc.sync.dma_start(
                out=hgall[:],
                in_=xexp3[:, t0 * K * F1:(t0 + G1) * K * F1])
            xgrp = work.tile([P, G1 * H], BF16, name="xgrp", tag="xgrp",
                             bufs=2)
            for ti in range(G1):
                t = t0 + ti
                pa = psum.tile([P, 4 * P], F32, name="pa", tag="pa", bufs=2)
                for c in range(K):
                    m = ti * K + c
                    mh = build_mh(t * K + c)
                    fs = range(kc1) if c < K - 1 else reversed(range(kc1))
                    for f in fs:
                        fw = min(P, F1 - f * P)
                        nc.tensor.matmul(
                            out=pa[:fw, f * P:(f + 1) * P],
                            lhsT=hgall[:, m * F1 + f * P:m * F1 + f * P + fw],
                            rhs=mh[:],
                            start=(c == 0 and f == 0),
                            stop=(c == K - 1 and f == 0),
                        )
                aggT = work.tile([P, kc1 * P], BF16, name="aggT1", tag="aggT1",
                                 bufs=2)
                nc.vector.tensor_copy(out=aggT[:, :P], in_=pa[:, :P])
                nc.vector.tensor_copy(out=aggT[:fw1, P:2 * P],
                                      in_=pa[:fw1, P:2 * P])
                prm = psum.tile([P, 4 * P], F32, name="prm", tag="pt", bufs=2)
                for k in range(kc1):
                    kw = min(P, F1 - k * P)
                    nc.tensor.matmul(
                        out=prm[:, :H],
                        lhsT=aggT[:kw, k * P:(k + 1) * P],
                        rhs=w1_sb[:kw, k * H:(k + 1) * H],
                        start=(k == 0),
                        stop=False,
                    )
                nc.tensor.matmul(
                    out=prm[:, :H],
                    lhsT=ones_sb[:, :],
                    rhs=b1row_sb[:, :],
                    start=False,
                    stop=True,
                )
                nc.scalar.activation(
                    out=xgrp[:, ti * H:(ti + 1) * H],
                    in_=prm[:, :H],
                    func=mybir.ActivationFunctionType.Relu,
                )
            nc.sync.dma_start(out=x2s3[:, t0:t0 + G1, :],
                              in_=xgrp[:].rearrange("p (t f) -> p t f", t=G1))
        allgather(x2s, x2f)
        dbg = cfg.get("dbg_aps") or {}
        if "dx2s" in dbg:
            nc.sync.dma_start(out=dbg["dx2s"], in_=x2s[:])
        if cfg.get("stages", 3) == 1:
            nc.vector.memset(outbuf[:], 0.0)
            nc.sync.dma_start(out=out_ap, in_=outbuf[:])
            return

        for t in range(T):
            hgs = []
            for c in range(K):
                hg = work.tile([P, H], BF16, name="hg2", tag="hg2", bufs=8)
                nc.gpsimd.indirect_dma_start(
                    out=hg[:],
                    out_offset=None,
                    in_=x2f[:],
                    in_offset=IndirectOffsetOnAxis(
                        ap=src_sb[:, t * K + c:t * K + c + 1], axis=0),
                )
                hgs.append(hg)
            pa = psum.tile([P, 4 * P], F32, name="pa2", tag="pa", bufs=2)
            for c in range(K):
                mh = build_mh(t * K + c)
                for f in range(kc2):
                    nc.tensor.matmul(
                        out=pa[:, f * P:(f + 1) * P],
                        lhsT=hgs[c][:, f * P:(f + 1) * P],
                        rhs=mh[:],
                        start=(c == 0 and f == 0),
                        stop=(c == K - 1 and f == kc2 - 1),
                    )
            aggT = work.tile([P, kc2 * P], BF16, name="aggT2", tag="aggT2",
                             bufs=2)
            nc.scalar.copy(out=aggT[:], in_=pa[:, :kc2 * P])
            pt = psum.tile([P, 4 * P], F32, name="pt", tag="pt", bufs=2)
            for j in range(npj):
                for k in range(kc2):
                    nc.tensor.matmul(
                        out=pt[:, j * P:(j + 1) * P],
                        lhsT=w2_sb[:, k * H + j * P:k * H + (j + 1) * P],
                        rhs=aggT[:, k * P:(k + 1) * P],
                        start=(j == 0 and k == 0),
                        stop=(j == npj - 1 and k == kc2 - 1),
                    )
            xT = work.tile([P, npj * P], BF16, name="xT", tag="xT", bufs=2)
            for j in range(npj - 1):
                nc.scalar.activation(
                    out=xT[:, j * P:(j + 1) * P],
                    in_=pt[:, j * P:(j + 1) * P],
                    func=mybir.ActivationFunctionType.Relu,
                    bias=b2_sb[:, j:j + 1],
                )
            j = npj - 1
            nc.vector.tensor_scalar(
                out=xT[:, j * P:(j + 1) * P],
                in0=pt[:, j * P:(j + 1) * P],
                scalar1=b2_sb[:, j:j + 1],
                scalar2=0.0,
                op0=mybir.AluOpType.add,
                op1=mybir.AluOpType.max,
            )
            h3p = psum.tile([P, 4 * P], F32, name="h3p", tag="h3p", bufs=2)
            for j in range(npj):
                nc.tensor.matmul(
                    out=h3p[:, :O],
                    lhsT=xT[:, j * P:(j + 1) * P],
                    rhs=w3_sb[:, j * O:(j + 1) * O],
                    start=(j == 0), stop=(j == npj - 1),
                )
            nc.scalar.copy(out=h3all[:, t * O:(t + 1) * O],
                           in_=h3p[:, :O])
        nc.sync.dma_start(out=h3s[:], in_=h3all[:])
        allgather(h3s, h3f)
        if "dh3s" in dbg:
            nc.sync.dma_start(out=dbg["dh3s"], in_=h3all[:])
        if cfg.get("stages", 3) == 2:
            nc.vector.memset(outbuf[:], 0.0)
            nc.sync.dma_start(out=out_ap, in_=outbuf[:])
            return

        h3rows = h3f[:].rearrange("p (t o) -> (p t) o", o=O)
        for t in range(T):
            hg3s = []
            for c in range(K):
                hg3 = work.tile([P, O], BF16, name="hg3", tag="hg3", bufs=8)
                nc.gpsimd.indirect_dma_start(
                    out=hg3[:],
                    out_offset=None,
                    in_=h3rows,
                    in_offset=IndirectOffsetOnAxis(
                        ap=src3_sb[:, t * K + c:t * K + c + 1], axis=0),
                )
                hg3s.append(hg3)
            po = psum.tile([P, 4 * P], F32, name="po", tag="h3p", bufs=2)
            for c in range(K):
                mh = build_mh(t * K + c)
                nc.tensor.matmul(
                    out=po[:, :O],
                    lhsT=mh[:],
                    rhs=hg3s[c][:],
                    start=(c == 0),
                    stop=(c == K - 1),
                )
            nc.vector.tensor_tensor(
                out=outbuf[:, t * O:(t + 1) * O],
                in0=po[:, :O],
                in1=b3_sb[:, t * O:(t + 1) * O],
                op=mybir.AluOpType.add,
            )
        nc.sync.dma_start(out=out_ap, in_=outbuf[:])



EXEC_NS = None


def _run_pjrt_timed(nc, in_maps, n_cores, time_iters=0):
    global EXEC_NS
    import jax
    import jax.numpy as jnp
    from jax.experimental.shard_map import shard_map
    from jax.sharding import Mesh, NamedSharding, PartitionSpec

    from concourse import bass2jax as b2j

    b2j.install_neuronx_cc_hook()

    partition_name = (nc.partition_id_tensor.name
                      if nc.partition_id_tensor else None)
    in_names, out_names, out_avals, zero_outs = [], [], [], []
    for alloc in nc.m.functions[0].allocations:
        if not isinstance(alloc, mybir.MemoryLocationSet):
            continue
        name = alloc.memorylocations[0].name
        if alloc.kind == "ExternalInput":
            if name != partition_name:
                in_names.append(name)
        elif alloc.kind == "ExternalOutput":
            out_names.append(name)
            shape = tuple(alloc.tensor_shape)
            dtype = mybir.dt.np(alloc.dtype)
            out_avals.append(jax.core.ShapedArray(shape, dtype))
            zero_outs.append(np.zeros(shape, dtype))
    n_params = len(in_names)
    all_in_names = list(in_names) + list(out_names)
    if partition_name is not None:
        all_in_names.append(partition_name)
    all_in_names = tuple(all_in_names)

    def _body(*args):
        operands = list(args)
        if partition_name is not None:
            operands.append(b2j.partition_id_tensor())
        outs = b2j._bass_exec_p.bind(
            *operands,
            out_avals=tuple(out_avals),
            in_names=all_in_names,
            out_names=tuple(out_names),
            lowering_input_output_aliases=(),
            sim_require_finite=True,
            sim_require_nnan=True,
            nc=nc,
        )
        return tuple(outs)

    devices = jax.devices()[:n_cores]
    assert len(devices) == n_cores
    mesh = Mesh(np.asarray(devices), ("core",))
    spec = PartitionSpec("core")
    n_all = n_params + len(zero_outs)
    jitted = jax.jit(shard_map(
        _body, mesh=mesh, in_specs=(spec,) * n_all,
        out_specs=(spec,) * len(out_names), check_rep=False))

    sharding = NamedSharding(mesh, spec)
    g_in = [
        jax.device_put(
            np.concatenate([np.asarray(in_maps[c][nm]) for c in range(n_cores)],
                           axis=0), sharding)
        for nm in in_names
    ]
    g_zero = [
        jax.device_put(np.concatenate([z] * n_cores, axis=0), sharding)
        for z in zero_outs
    ]

    out_arrs = jitted(*g_in, *g_zero)
    jax.block_until_ready(out_arrs)
    results = [
        {nm: np.asarray(out_arrs[i]).reshape(n_cores, *out_avals[i].shape)[c]
         for i, nm in enumerate(out_names)}
        for c in range(n_cores)
    ]

    if time_iters > 0:
        triv = jax.jit(shard_map(
            lambda a: (a + 1.0,), mesh=mesh, in_specs=(spec,),
            out_specs=(spec,), check_rep=False))
        tiny = jax.device_put(np.zeros((n_cores * 8, 8), np.float32), sharding)
        jax.block_until_ready(triv(tiny))
        walls, base = [], []
        for _ in range(time_iters):
            t0 = time.perf_counter()
            o = jitted(*g_in, *g_zero)
            jax.block_until_ready(o)
            walls.append(time.perf_counter() - t0)
            t0 = time.perf_counter()
            o = triv(tiny)
            jax.block_until_ready(o)
            base.append(time.perf_counter() - t0)
        walls = np.array(walls)
        base = np.array(base)
        diffs = walls - base
        EXEC_NS = int((np.min(walls) - np.min(base)) * 1e9)
        print(f"[timing] kernel min {np.min(walls)*1e3:.3f} "
              f"med {np.median(walls)*1e3:.3f} ms | base min "
              f"{np.min(base)*1e3:.3f} med {np.median(base)*1e3:.3f} ms | "
              f"min-diff {EXEC_NS/1e3:.0f} us  med-diff "
              f"{np.median(diffs)*1e6:.0f} us")
    return results



def prepare(x, edge_index, W1, b1, W2, b2, W3, b3):
    import ml_dtypes
    x = np.asarray(x, np.float32)
    edge_index = np.asarray(edge_index)
    n_nodes = x.shape[0]
    assert n_nodes == N_NODES and x.shape[1] == F_IN

    pre = _preprocess(edge_index, n_nodes, N_CORES, TILES_PER_CORE)
    T = TILES_PER_CORE
    cfg = dict(n_cores=N_CORES, T=T, K=pre["K"], F1=F_IN, H=HIDDEN, O=F_OUT)

    x_dev = np.zeros((pre["n_pad"], F_IN), ml_dtypes.bfloat16)
    x_dev[pre["perm"][:n_nodes]] = x

    common = dict(
        iota=np.tile(np.arange(P, dtype=np.float32), (P, 1)).copy(),
        w1=_pack_w(W1, HIDDEN),
        b1row=np.asarray(b1, np.float32).reshape(1, HIDDEN)
              .astype(ml_dtypes.bfloat16).copy(),
        b2=_pack_b(b2),
        w2=_pack_w(W2, HIDDEN),
        w3=_pack_w(W3, F_OUT),
        b3full=np.tile(np.asarray(b3, np.float32), (P, T)).copy(),
    )
    in_maps = []
    for c in range(N_CORES):
        m = dict(common)
        m["src"] = pre["src"][c]
        m["dstf"] = pre["dstf"][c]
        m["nrm"] = pre["nrm"][c]
        m["src3"] = pre["src3"][c]
        m["xexp"] = x_dev[pre["src"][c]].reshape(P, -1).copy()
        in_maps.append(m)
    return pre, cfg, in_maps


def kernel(x, edge_index, W1, b1, W2, b2, W3, b3, _trace=False, _time_iters=0):
    global LAST_RESULTS
    pre, cfg, in_maps = prepare(x, edge_index, W1, b1, W2, b2, W3, b3)
    T = TILES_PER_CORE
    n_pad = pre["n_pad"]
    n_nodes = N_NODES

    nc = bacc.Bacc("TRN2", target_bir_lowering=False, debug=False,
                   enable_asserts=False, num_devices=N_CORES)
    ins_aps = {}
    for name, arr in in_maps[0].items():
        ins_aps[name] = nc.dram_tensor(
            name, list(arr.shape), mybir.dt.from_np(arr.dtype),
            kind="ExternalInput").ap()
    out_t = nc.dram_tensor("out", [P, T * F_OUT], F32, kind="ExternalOutput")

    with tile.TileContext(nc) as tc:
        _build_gcn(tc, ins_aps, out_t.ap(), cfg)
    nc.compile()

    if axon_active():
        results = _run_pjrt_timed(nc, in_maps, N_CORES, time_iters=_time_iters)
    else:
        res = bass_utils.run_bass_kernel_spmd(
            nc, in_maps, core_ids=list(range(N_CORES)), trace=_trace)
        LAST_RESULTS = res
        results = res.results

    out_dev = np.zeros((n_pad, F_OUT), np.float32)
    for c in range(N_CORES):
        o = results[c]["out"]
        rows = o.reshape(P, T, F_OUT).transpose(1, 0, 2).reshape(T * P, F_OUT)
        out_dev[c * T * P:(c + 1) * T * P] = rows
    return out_dev[pre["perm"][:n_nodes]].copy()
